# revision 1
# baseline (speedup 1.0000x reference)
"""Trainium2 Bass kernel for nn_DendSN (dendritic spiking neuron layer).

Math (per element):
  pre[t]  = sum_{s<=t} alpha^(t-s) x[s]                (temporal EMA, T=8)
  y[t]    = sum_b softmax(k)_b * 3 * mexican_hat(pre_b)  (B=4 compartments)
  LIF:  h = beta*h + y ; spike = (h >= 1) ; h = spike ? 0 : h

Kernel strategy (8 cores, data-parallel over batch N=16 -> 2 per core):
  * temporal EMA as a PE matmul with a block-diagonal decay matrix BD:
      BD[(g,s),(g',t)] = (g==g') * alpha^(t-s);  16 independent element
      groups g ride the contraction dim so the 8-wide T-contraction still
      uses all 128 PE rows.
  * mexican hat via ONE ScalarE Derivative_Erf pass (= a Gaussian of the
    input) plus one fused custom-DVE op (1 - pre^2/v)*g (MEXICAN_HAT_ANT,
    registered at runtime into the custom-DVE table).
  * compartment reduction as a second PE matmul: the mexican-hat tile is
    the stationary operand, streamed against a small constant matrix W2
    that folds softmax(k), the mexican-hat constants and the LIF rescale
    2^t. The PE instruction stream is software-pipelined (mm2 of tile q-1
    is emitted after mm1 of tile q) so the in-order PE never stalls on the
    ACT->DVE mexican-hat latency.
  * LIF runs in the scaled space h~ = h * 2^t (beta = 0.5), which makes the
    recurrence a pure add and is bit-exact vs the unscaled recurrence
    (scaling by powers of two is exact in fp32). Threshold at t is 2^t.
    One fused custom-DVE op per step: h' = select(h + y < 2^t, h + y, 0);
    spikes are recovered off the critical chain on GPSIMD as (h' == 0) and
    written as bf16 (0/1 exact), streamed out per t-pair.
"""

import numpy as np

T, N_FULL, C, H, W = 8, 16, 128, 32, 32
L = H * W
B = 4
K = C // B
N_CORES = 8
NSH = N_FULL // N_CORES  # batches per core = 2
ALPHA = 0.5
VAR = 0.75 + 1e-5
NQ = 8  # element tiles per batch; each covers 16 channels * L = 16384 elems

_CACHE = {}
# f32r mm1 runs 4x faster on PE but costs ~10x output error
# (188 vs 2 flipped spikes measured on HW); keep full fp32.
MM1_F32R = False


def _build_constants(k_np):
    import math

    idx = np.arange(T)
    d = idx[:, None] - idx[None, :]
    A = np.where(d >= 0, ALPHA ** np.maximum(d, 0).astype(np.float64), 0.0)
    # row p = s*16 + g ; col m = g*8 + t ; value A[t, s] on matching g
    BD = np.zeros((128, 128), dtype=np.float64)
    for g in range(16):
        for s in range(T):
            for t in range(T):
                BD[s * 16 + g, g * 8 + t] = A[t, s]

    kk = np.asarray(k_np, dtype=np.float64).reshape(B)
    ek = np.exp(kk - kk.max())
    fs = ek / ek.sum()
    # Derivative_Erf yields (2/sqrt(pi)) * exp(-x^2); fold the sqrt(pi)/2
    # normalization plus the mexican-hat constants into the W2 weights.
    wgt = fs * 3.0 / (VAR * math.sqrt(2.0 * math.pi * VAR))
    wgt = wgt * (math.sqrt(math.pi) / 2.0)
    W2 = np.zeros((128, 32), dtype=np.float64)
    for gp in range(16):  # gp = kcrel*4 + b
        kcrel, b = gp // 4, gp % 4
        for t in range(T):
            W2[gp * 8 + t, t * 4 + kcrel] = wgt[b] * float(2.0 ** t)
    return BD.astype(np.float32), W2.astype(np.float32)


_OPS_REGISTERED = {}


def _register_custom_ops():
    """Register fused DVE ops (mexican hat, LIF step, spike) at runtime.

    The custom-DVE table is generated per-NEFF from dve_ops.OPS, the opcode
    row from _SUB_OPCODE_FOR_NAME, and CoreSim reads CUSTOM_DVE_SPECS — all
    Python-side registries, so appending here flows through trace, compile
    and simulation.
    """
    if _OPS_REGISTERED:
        return _OPS_REGISTERED
    import numpy as _np
    import concourse.dve_ops as dve_ops
    from concourse.dve_spec import Spec, Src0, Src1, C0, C1, Zero, sq, select, lower
    from concourse.dve_spec import _has_src1 as has_src1
    from concourse.dve_uop import DveOpSpec
    from concourse.dve_table_gen import dve_ver_for

    ver = dve_ver_for("TRN2")

    def mk(name, spec):
        if name in dve_ops._SUB_OPCODE_FOR_NAME:  # re-import of this module
            return next(op for op in dve_ops.OPS if op.name == name)
        row = 1 + len(dve_ops.OPS)
        assert row < 0x20
        compiled = DveOpSpec(
            name=name, opcode=row, uops=lower(spec, ver=ver),
            rd1_en=has_src1(spec),
        )
        op = dve_ops.DveOp(name, spec, False, {ver: compiled.sha(ver)})
        dve_ops.OPS.append(op)
        dve_ops._SUB_OPCODE_FOR_NAME[name] = row
        dve_ops.CUSTOM_DVE_SPECS[name] = spec
        return op

    def _ref_mh(in0, in1, c0, c1, c2):
        x = in0.astype(_np.float32)
        return (c0 - (x * c1) ** 2) * in1.reshape(in0.shape)

    def _ref_lif(in0, in1, c0, c1, c2):
        e = in0.astype(_np.float32) + in1.reshape(in0.shape)
        return _np.where(e < c0, e, 0.0).astype(_np.float32)

    def _ref_spike(in0, in1, c0, c1, c2):
        e = in0.astype(_np.float32) + in1.reshape(in0.shape)
        return (e >= c0).astype(_np.float32)

    # out = (c0 - (in0*c1)^2) * in1   -- (1 - pre^2/v) * gaussian
    _OPS_REGISTERED["mh"] = mk(
        "MEXICAN_HAT_ANT",
        Spec(body=(C0 - sq(Src0 * C1)) * Src1, reference=_ref_mh),
    )
    _e = Src0 + Src1
    # out = select(h + y < th, h + y, 0)  -- LIF accumulate + hard reset
    _OPS_REGISTERED["lif"] = mk(
        "LIF_STEP_ANT",
        Spec(body=select(_e < C0, _e, Zero), reference=_ref_lif),
    )
    # out = (h + y >= th)  -- spike output
    _OPS_REGISTERED["spike"] = mk(
        "SPIKE_ANT",
        Spec(body=_e >= C0, reference=_ref_spike),
    )
    return _OPS_REGISTERED


def _build_program(k_np):
    import concourse.bacc as bacc
    import concourse.tile as tile
    from concourse import mybir

    f32 = mybir.dt.float32
    ops = _register_custom_ops()
    BD, W2 = _build_constants(k_np)

    nc = bacc.Bacc("TRN2", target_bir_lowering=False, debug=False)
    x_in = nc.dram_tensor("x", [T, NSH, C, H, W], f32, kind="ExternalInput")
    # Device-native output layout: [t, n, partition m, col f] with
    # f = (q*8+c8)*4 + kcrel ; host un-permutes to (kc, h, w). Spikes are
    # exactly 0.0/1.0 so bf16 is lossless and halves the store traffic.
    bf16 = mybir.dt.bfloat16
    out = nc.dram_tensor("out", [T, NSH, 128, 256], bf16, kind="ExternalOutput")
    bd_d = nc.inline_tensor(BD, "bd")
    w2_d = nc.inline_tensor(W2, "w2")

    # DRAM views.
    # x element (s, n, flat) with flat = q*16384 + gg*1024 + j lives at SBUF
    # partition s*16+gg, col q*1024+j.  (gg = channel within tile, j = lambda)
    xv = x_in[:].rearrange("t n c h w -> t n (c h w)")
    xv = xv.rearrange("t n (q gg j) -> n q t gg j", q=NQ, gg=16, j=1024)
    # out element (t, n, m, f) <- stage partition m, col t*256+f
    ov = out[:].rearrange("t n m f -> n m t f")

    derf_scale = float(1.0 / np.sqrt(2.0 * VAR))
    mh_c1 = float(1.0 / np.sqrt(VAR))

    with tile.TileContext(nc) as tc:
        with (
            tc.tile_pool(name="singles", bufs=1) as singles,
            tc.tile_pool(name="xbuf", bufs=2) as xbuf,
            tc.tile_pool(name="act", bufs=5) as actp,
            tc.tile_pool(name="mh", bufs=5) as mhp,
            tc.tile_pool(name="hbuf", bufs=3) as hbuf,
            tc.tile_pool(name="stage", bufs=2) as stagep,
            tc.tile_pool(name="pre", bufs=4, space="PSUM") as prep,
            tc.tile_pool(name="ybank", bufs=1, space="PSUM") as ybankp,
        ):
            mm1_dt = mybir.dt.float32r if MM1_F32R else f32
            bd_s = singles.tile([128, 128], mm1_dt)
            w2_s = singles.tile([128, 32], f32)
            if MM1_F32R:
                nc.gpsimd.dma_start(bd_s[:], bd_d[:])  # SWDGE casts f32->f32r
            else:
                # constants ride the ScalarE HWDGE queue so the x loads
                # start immediately on the SP queue
                nc.scalar.dma_start(bd_s[:], bd_d[:])
            nc.scalar.dma_start(w2_s[:], w2_d[:])

            for n in range(NSH):
                xs = xbuf.tile([128, NQ, 1024], mm1_dt)
                dma = nc.gpsimd.dma_start if MM1_F32R else nc.sync.dma_start
                for q in range(NQ):  # ~1 MiB DMAs, full 128 partitions
                    if n == 0 and q == 0:
                        # split the first chunk so mm1 can start sooner
                        for c4 in range(4):
                            dma(xs[:, q, c4 * 256:(c4 + 1) * 256],
                                xv[n, q, :, :, c4 * 256:(c4 + 1) * 256])
                    else:
                        dma(xs[:, q, :], xv[n, q])

                yb = ybankp.tile([128, 2048], f32, tag="ybank", name=f"yb{n}")
                # col = qc*32 + t*4 + kcrel
                yv = yb.rearrange("p (qc t kcrel) -> p qc t kcrel", qc=64, t=T)

                def emit_mm2(q, mh):
                    for c8 in range(8):
                        qc = q * 8 + c8
                        nc.tensor.matmul(
                            yb[:, qc * 32:(qc + 1) * 32],
                            mh[:, c8 * 128:(c8 + 1) * 128],
                            w2_s[:],
                            start=(qc % 16 == 0), stop=(qc % 16 == 15),
                            skip_group_check=True,
                        )

                pending = None  # software-pipeline: mm2(q-1) after mm1(q)
                for q in range(NQ):
                    mh = mhp.tile([128, 1024], f32)
                    for i in range(2):
                        pre = prep.tile([128, 512], f32, tag="pre",
                                        name=f"pre{n}_{q}_{i}")
                        if n == 0 and q == 0:
                            # quarter-granular so the first matmuls start as
                            # soon as each quarter-DMA lands
                            for ii in range(2):
                                nc.tensor.matmul(
                                    pre[:, ii * 256:(ii + 1) * 256], bd_s[:],
                                    xs[:, q, i * 512 + ii * 256:
                                       i * 512 + (ii + 1) * 256],
                                    start=(ii == 0), stop=(ii == 1),
                                    skip_group_check=True,
                                )
                        else:
                            nc.tensor.matmul(
                                pre[:], bd_s[:],
                                xs[:, q, i * 512:(i + 1) * 512],
                                start=True, stop=True,
                            )
                        g = actp.tile([128, 512], f32, tag="g")
                        # g = (2/sqrt(pi)) * exp(-pre^2 / (2*VAR))
                        nc.scalar.activation(
                            g[:], pre[:],
                            mybir.ActivationFunctionType.Derivative_Erf,
                            scale=derf_scale,
                        )
                        # mh = (1 - pre^2/VAR) * g
                        nc.vector._custom_dve(
                            ops["mh"], out=mh[:, i * 512:(i + 1) * 512],
                            in0=pre[:], in1=g[:], s0=1.0, s1=mh_c1,
                        )
                    if pending is not None:
                        emit_mm2(q - 1, pending)
                    pending = mh
                emit_mm2(NQ - 1, pending)

                stage = stagep.tile([128, 2048], bf16)
                stv = stage.rearrange("p (t f) -> p t f", t=T)
                h = hbuf.tile([128, 256], f32, tag="h0", name=f"h0_{n}")
                nc.gpsimd.memset(h[:], 0.0)
                for t in range(T):
                    th = float(2.0 ** t)
                    ysl = yv[:, :, t, :]
                    if t == T - 1:
                        # last step: no h needed downstream; emit the spike
                        # directly from (h + y >= th) on DVE (one fewer hop)
                        nc.vector._custom_dve(
                            ops["spike"], out=stage[:, t * 256:(t + 1) * 256],
                            in0=h[:], in1=ysl, s0=th,
                        )
                        nc.sync.dma_start(
                            ov[n, :, t - 1:t + 1, :], stv[:, t - 1:t + 1, :]
                        )
                        break
                    hn = hbuf.tile([128, 256], f32, tag="h", name=f"h_{n}_{t}")
                    # h' = select(h + y < th, h + y, 0)  (DVE, serial chain)
                    nc.vector._custom_dve(
                        ops["lif"], out=hn[:], in0=h[:], in1=ysl, s0=th,
                    )
                    h = hn
                    # spike = (h' == 0): post-reset h is 0 iff spiked
                    # (exact-zero h+y collision ~1e-7 probability).
                    # GPSIMD keeps spikes off the critical DVE chain.
                    nc.gpsimd.tensor_scalar(
                        stage[:, t * 256:(t + 1) * 256],
                        hn[:], 0.0, None, mybir.AluOpType.is_equal,
                    )
                    if t % 2 == 1:  # stream spikes out per t-pair
                        nc.sync.dma_start(
                            ov[n, :, t - 1:t + 1, :], stv[:, t - 1:t + 1, :]
                        )

    nc.compile()
    return nc


def _get_program(k_np):
    key = np.asarray(k_np, dtype=np.float32).tobytes()
    if key not in _CACHE:
        _CACHE[key] = _build_program(k_np)
    return _CACHE[key]


def kernel(x_seq, k, _want_trace=False):
    from concourse import bass_utils

    x_np = np.ascontiguousarray(np.asarray(x_seq, dtype=np.float32))
    k_np = np.asarray(k, dtype=np.float32)
    nc = _get_program(k_np)

    in_maps = [
        {"x": np.ascontiguousarray(x_np[:, c * NSH:(c + 1) * NSH])}
        for c in range(N_CORES)
    ]
    res = bass_utils.run_bass_kernel_spmd(
        nc, in_maps, core_ids=list(range(N_CORES)), trace=_want_trace,
    )
    out = np.concatenate([_unpermute(r["out"]) for r in res.results], axis=1)
    if _want_trace:
        kernel._last_results = res
    return out


def _unpermute(dev_out):
    """[T, NSH, 128(m), 256(f)] -> [T, NSH, K, H, W].

    f = (q*8+c8)*4 + kcrel ; kc = q*4 + kcrel ; l = c8*128 + m.
    """
    a = np.asarray(dev_out, dtype=np.float32)
    a = a.reshape(T, NSH, 128, NQ, 8, 4)  # t n m q c8 kcrel
    a = a.transpose(0, 1, 3, 5, 4, 2)           # t n q kcrel c8 m
    return np.ascontiguousarray(a).reshape(T, NSH, K, H, W)

